# revision 10
# baseline (speedup 1.0000x reference)
"""Trainium2 Bass kernel for nn_LstmEncoder: two 5-layer LSTM stacks.

Architecture (hardcoded from the problem spec):
  x [256, 256, 8] -> stack1 (F=8 -> H=256, 5 layers) -> stack2 (H=256 -> E=128,
  5 layers) -> returns final hidden of last layer, [256, 128].

Sharding: data-parallel over batch, 32 rows per core on 8 cores; weights
replicated.  Per core, the 10 LSTM layers run as a diagonal wavefront
(layer q processes timestep t = s - q at wavefront step s); 4 layers pack
into the 4 PE column groups (tile_position col-tiling, batch=32 per group).

All matmul operands stream in bf16 (1 cycle/row on TRN2 vs 4 for fp32);
psum accumulation and the cell state c stay fp32.

Layouts per core:
  - Pass A psum [128, 1024]: rows 32q..32q+32 = stack1 layer q (q=0..3),
    cols [I|F|O|G] x 256.
  - Pass B psum [128, 1024]: rows 0:32 = q4 (Hd=256, cols [I|F|O|G]x256);
    rows 32:64 = q5 (slot0); rows 64:96 = q6(slot0)+q7(slot1);
    rows 96:128 = q8(slot0)+q9(slot1).  Stack2 slot s puts gate block
    g in cols 256*g + 128*s, so sigmoid covers [0:768], tanh [768:1024].
  - h [128, 512] bf16; hT via PE transposes (bf16, 1cyc/row) of the four
    128-col blocks -> psum -> DVE copy to SBUF as matmul stationaries.
  - Emission per step: TA(s-1) -> phase1 MMs (pass A + q4-input) ->
    TB(s-1) -> phase2 MMs (pass B recurrents + stack2) so ACT/DVE work of
    one pass hides under the other pass's matmuls.
"""

import numpy as np
import ml_dtypes

B, T_FULL, F, E = 256, 256, 8, 128
H = 2 * E          # 256
NL = 5
NCORES = 8
BSH = B // NCORES  # 32

BF16 = ml_dtypes.bfloat16


def _gate_perm(Hd):
    """PyTorch gate order [i f g o] -> our column order [i f o g]."""
    return np.concatenate([
        np.arange(0, Hd),
        np.arange(Hd, 2 * Hd),
        np.arange(3 * Hd, 4 * Hd),
        np.arange(2 * Hd, 3 * Hd),
    ])


# ---------------- layer table (chain index q = 0..9) ----------------
# q 0..4: stack1 (Hd=256, gate row 1024); q 5..9: stack2 (Hd=128, 512).

def _is_s1(q):
    return q < 5


def _pass_of(q):
    return "A" if q < 4 else "B"


def _rows(q):
    if q < 4:
        return 32 * q, 32 * q + 32
    r = {4: 0, 5: 32, 6: 64, 7: 64, 8: 96, 9: 96}[q]
    return r, r + 32


def _slot(q):
    # column slot within the 128-wide gate sub-blocks (stack2 only)
    return {5: 0, 6: 0, 7: 1, 8: 0, 9: 1}.get(q, 0)


def _ch_cols(q):
    """(c/h col range) within c_all/h_all [128, 512]."""
    if q < 4:
        return 0, 256
    if q == 4:
        return 256, 512
    return (256, 384) if _slot(q) == 0 else (384, 512)


# hT geometry: TA0/TA1 hold stack1 feats 0:128 / 128:256 (cols = 32q+b for
# pass-A layer q); TB0/TB1 hold pass-B feats (h cols 256:384 / 384:512,
# cols = row index of the layer).
def _own_hT(q):
    if q < 4:
        return [("A0", 32 * q), ("A1", 32 * q)]
    return {4: [("B0", 0), ("B1", 0)],
            5: [("B0", 32)],
            6: [("B0", 64)],
            7: [("B1", 64)],
            8: [("B0", 96)],
            9: [("B1", 96)]}[q]


def _in_hT(q):
    if q == 0:
        return None
    if q <= 4:
        return [("A0", 32 * (q - 1)), ("A1", 32 * (q - 1))]
    return {5: [("B0", 0), ("B1", 0)],
            6: [("B0", 32)],
            7: [("B0", 64)],
            8: [("B1", 64)],
            9: [("B0", 96)]}[q]


def _colgroup(q):
    """PE column group == psum row block of the layer."""
    return _rows(q)[0] // 32


def build_nc(T, smax=None):
    import concourse.bass as bass
    import concourse.mybir as mybir
    import concourse.tile as tile
    from concourse import bacc
    from contextlib import ExitStack

    fp = mybir.dt.float32
    bf = mybir.dt.bfloat16
    AF = mybir.ActivationFunctionType
    nc = bacc.Bacc("TRN2", target_bir_lowering=False)

    # ---------------- DRAM declarations ----------------
    xT_d = nc.dram_tensor("xT", [9, T * BSH], bf, kind="ExternalInput")
    id_d = nc.dram_tensor("ident", [128, 128], bf, kind="ExternalInput")
    on_d = nc.dram_tensor("ones", [1, 32], bf, kind="ExternalInput")
    win_d, whh_d, b_d = {}, {}, {}
    for q in range(10):
        G = 1024 if _is_s1(q) else 512
        kt_in = 1 if q == 0 else (2 if (_is_s1(q) or q == 5) else 1)
        kt_hh = 2 if _is_s1(q) else 1
        for k in range(kt_in):
            kp = 9 if q == 0 else 128
            win_d[q, k] = nc.dram_tensor(f"win{q}_{k}", [kp, G], bf,
                                         kind="ExternalInput")
        for k in range(kt_hh):
            whh_d[q, k] = nc.dram_tensor(f"whh{q}_{k}", [128, G], bf,
                                         kind="ExternalInput")
    # biases: stack1 layers 1-4 own [1,1024]; q5 solo; (q6,q7), (q8,q9)
    # combined into [1,1024] rows (slot packing matches gate layout).
    for q in range(1, 6):
        b_d[q] = nc.dram_tensor(f"bias{q}", [1, 1024], bf, kind="ExternalInput")
    b_d[6] = nc.dram_tensor("bias6", [1, 1024], bf, kind="ExternalInput")
    b_d[8] = nc.dram_tensor("bias8", [1, 1024], bf, kind="ExternalInput")
    out_d = nc.dram_tensor("out", [BSH, E], fp, kind="ExternalOutput")

    with tile.TileContext(nc) as tc, ExitStack() as ctx:
        wpool = ctx.enter_context(tc.tile_pool(name="weights", bufs=1))
        state = ctx.enter_context(tc.tile_pool(name="state", bufs=1))
        hpool = ctx.enter_context(tc.tile_pool(name="hpool", bufs=2))
        hTpool = ctx.enter_context(tc.tile_pool(name="hTpool", bufs=2))
        apool = ctx.enter_context(tc.tile_pool(name="apool", bufs=2))
        gApool = ctx.enter_context(tc.tile_pool(name="gApool", bufs=2,
                                                space="PSUM"))
        gBpool = ctx.enter_context(tc.tile_pool(name="gBpool", bufs=1,
                                                space="PSUM"))
        tppool = ctx.enter_context(tc.tile_pool(name="tppool", bufs=2,
                                                space="PSUM"))

        def load(dram, dt=bf):
            t = wpool.tile(list(dram.shape), dt, name=f"sb_{dram.name}")
            nc.sync.dma_start(t[:], dram[:])
            return t

        xT_sb = load(xT_d)
        id_sb = load(id_d)
        on_sb = load(on_d)
        win_sb = {k: load(v) for k, v in win_d.items()}
        whh_sb = {k: load(v) for k, v in whh_d.items()}
        b_sb = {k: load(v) for k, v in b_d.items()}

        c_all = state.tile([128, 512], fp, name="c_all")
        nc.gpsimd.memset(c_all[:], 0.0)
        hfin = state.tile([128, 128], fp, name="hfin")

        prev_hT = None
        h_tile = None

        SMAX = (T + 9 if smax is None else smax + 1)
        for s in range(SMAX):
            act = [q for q in range(10) if 0 <= s - q <= T - 1]
            actA = [q for q in act if _pass_of(q) == "A"]
            actB = [q for q in act if _pass_of(q) == "B"]

            for q in act:
                if s - q == 0:
                    r0, r1 = _rows(q)
                    cc0, cc1 = _ch_cols(q)
                    nc.gpsimd.memset(c_all[r0:r1, cc0:cc1], 0.0)

            g_tiles = {}
            if actA:
                g_tiles["A"] = gApool.tile([128, 1024], fp, name="gA", tag="gA")
            if actB:
                g_tiles["B"] = gBpool.tile([128, 1024], fp, name="gB", tag="gB")

            # ---------- build matmul sequences ----------
            # mms[q] = list of (out, lhsT, rhs, start, stop) in 2-region
            # interleave; phase1 = pass A + q4-input/bias; phase2 = rest.
            def hT_ap(key, col):
                return cur_hT[key][:, col:col + 32]

            def layer_mms(q, parts):
                """parts subset of {'in','rec','bias'} -> interleaved seq."""
                t = s - q
                g = g_tiles[_pass_of(q)]
                r0, r1 = _rows(q)
                mms_r = {0: [], 1: []}
                if _is_s1(q):
                    if "bias" in parts and q >= 1:
                        for r, (n0, n1) in enumerate(((0, 512), (512, 1024))):
                            mms_r[r].append((g[r0:r1, n0:n1], on_sb[0:1, 0:32],
                                             b_sb[q][0:1, n0:n1], True))
                    if "in" in parts:
                        kt_in = 1 if q == 0 else 2
                        for k in range(kt_in):
                            if q == 0:
                                lh = xT_sb[:, 32 * t:32 * t + 32]
                            else:
                                bk, col = _in_hT(q)[k]
                                lh = hT_ap(bk, col)
                            for r, (n0, n1) in enumerate(((0, 512),
                                                          (512, 1024))):
                                mms_r[r].append((g[r0:r1, n0:n1], lh,
                                                 win_sb[q, k][:, n0:n1],
                                                 q == 0 and k == 0))
                    if "rec" in parts and t > 0:
                        for k in range(2):
                            bk, col = _own_hT(q)[k]
                            lh = hT_ap(bk, col)
                            for r, (n0, n1) in enumerate(((0, 512),
                                                          (512, 1024))):
                                mms_r[r].append((g[r0:r1, n0:n1], lh,
                                                 whh_sb[q, k][:, n0:n1],
                                                 False))
                else:
                    slot = _slot(q)
                    if "bias" in parts:
                        # q5 solo; q6 emits for (q6,q7); q8 for (q8,q9).
                        bq = {5: 5, 6: 6, 7: 6, 8: 8, 9: 8}[q]
                        first = (q == min(x for x in act
                                          if x >= 4 and _rows(x) == (r0, r1)))
                        if first and bq in b_sb:
                            for r, (n0, n1) in enumerate(((0, 512),
                                                          (512, 1024))):
                                mms_r[r].append((g[r0:r1, n0:n1],
                                                 on_sb[0:1, 0:32],
                                                 b_sb[bq][0:1, n0:n1], True))
                    if "in" in parts:
                        kt_in = 2 if q == 5 else 1
                        for k in range(kt_in):
                            bk, col = _in_hT(q)[k]
                            lh = hT_ap(bk, col)
                            for c in range(4):
                                o0 = 256 * c + 128 * slot
                                mms_r[c // 2].append(
                                    (g[r0:r1, o0:o0 + 128], lh,
                                     win_sb[q, k][:, 128 * c:128 * c + 128],
                                     False))
                    if "rec" in parts and t > 0:
                        bk, col = _own_hT(q)[0]
                        lh = hT_ap(bk, col)
                        for c in range(4):
                            o0 = 256 * c + 128 * slot
                            mms_r[c // 2].append(
                                (g[r0:r1, o0:o0 + 128], lh,
                                 whh_sb[q, 0][:, 128 * c:128 * c + 128],
                                 False))
                seq = []
                for i in range(max(len(mms_r[0]), len(mms_r[1]))):
                    for r in range(2):
                        if i < len(mms_r[r]):
                            o, lh, rh, st = mms_r[r][i]
                            last = i == len(mms_r[r]) - 1
                            seq.append((o, lh, rh, st, last))
                return seq

            def emit_round_robin(groups):
                maxlen = max((len(x) for x in groups.values()), default=0)
                for i in range(maxlen):
                    for j in range(4):
                        if i < len(groups[j]):
                            o, lh, rh, st, sp = groups[j][i]
                            nc.tensor.matmul(o, lh, rh, start=st, stop=sp,
                                             skip_group_check=True,
                                             tile_position=(0, 32 * j))

            # ---------- transposes of h(s-1) ----------
            # TA feeds phase1; TB feeds phase2.  First step has no h.
            def emit_transposes(which):
                tpt = tp_tile
                for ki in which:
                    src = h_tile[:, 128 * ki:128 * ki + 128]
                    nc.tensor.transpose(tpt[:, 128 * ki:128 * ki + 128], src,
                                        id_sb[:])
                    key = ("A0", "A1", "B0", "B1")[ki]
                    dst = hTpool.tile([128, 128], bf, name=f"hT{key}",
                                      tag=f"hT{key}")
                    nc.vector.tensor_copy(dst[:],
                                          tpt[:, 128 * ki:128 * ki + 128])
                    cur_hT[key] = dst

            cur_hT = {}
            tp_tile = tppool.tile([128, 512], bf, name="tp", tag="tp") \
                if h_tile is not None else None
            # TA feeds pass A (q0-3 rec, q1-3 in) and q4's input; TB feeds
            # the rest.  Emit only when h(s-1) has the data and someone
            # consumes it.
            prevA = h_tile is not None and any(0 <= (s - 1) - q <= T - 1
                                               for q in range(0, 4)) \
                and (actA or 4 in actB)
            prevB = h_tile is not None and any(0 <= (s - 1) - q <= T - 1
                                               for q in range(4, 10)) \
                and actB
            if prev_hT:
                cur_hT.update(prev_hT)

            # NOTE on start/stop flags: with the per-layer part split, the
            # "stop" marker of phase1 q4-input seq is not the real group end
            # (rec comes in phase2); skip_group_check makes them advisory.
            if prevA:
                emit_transposes((0, 1))
            groups1 = {j: [] for j in range(4)}
            for q in actA:
                groups1[_colgroup(q)].extend(layer_mms(q, ("bias", "in", "rec")))
            if 4 in actB:
                groups1[_colgroup(4)].extend(layer_mms(4, ("bias", "in")))
            emit_round_robin(groups1)

            if prevB:
                emit_transposes((2, 3))
            groups2 = {j: [] for j in range(4)}
            if 4 in actB:
                groups2[_colgroup(4)].extend(layer_mms(4, ("rec",)))
            for q in actB:
                if q != 4:
                    groups2[_colgroup(q)].extend(
                        layer_mms(q, ("bias", "in", "rec")))
            emit_round_robin(groups2)
            prev_hT = cur_hT

            # ---------- activations + state update, per pass ----------
            new_h = hpool.tile([128, 512], bf, name="h", tag="h")
            for pas, qs in (("A", actA), ("B", actB)):
                if not qs:
                    continue
                g = g_tiles[pas]
                lo = min(_rows(q)[0] for q in qs)
                hi = max(_rows(q)[1] for q in qs)
                segs = [(32, 64), (64, hi)] if (lo == 32 and hi > 64) \
                    else [(lo, hi)]
                c0 = 0 if pas == "A" else 256
                s_ifo = apool.tile([128, 768], fp, name=f"sifo{pas}",
                                   tag=f"sifo{pas}")
                s_g = apool.tile([128, 256], bf, name=f"sg{pas}",
                                 tag=f"sg{pas}")
                tmp1 = apool.tile([128, 256], fp, name=f"tmp1{pas}",
                                  tag=f"tmp1{pas}")
                tmp2 = apool.tile([128, 256], fp, name=f"tmp2{pas}",
                                  tag=f"tmp2{pas}")
                thc = apool.tile([128, 256], bf, name=f"thc{pas}",
                                 tag=f"thc{pas}")
                for lo, hi in segs:
                    nc.scalar.activation(s_ifo[lo:hi, :], g[lo:hi, 0:768],
                                         AF.Sigmoid)
                    nc.scalar.activation(s_g[lo:hi, :], g[lo:hi, 768:1024],
                                         AF.Tanh)
                    nc.vector.tensor_mul(tmp1[lo:hi, :], s_ifo[lo:hi, 256:512],
                                         c_all[lo:hi, c0:c0 + 256])
                    nc.vector.tensor_mul(tmp2[lo:hi, :], s_ifo[lo:hi, 0:256],
                                         s_g[lo:hi, :])
                    nc.vector.tensor_add(c_all[lo:hi, c0:c0 + 256],
                                         tmp1[lo:hi, :], tmp2[lo:hi, :])
                    nc.scalar.activation(thc[lo:hi, :],
                                         c_all[lo:hi, c0:c0 + 256], AF.Tanh)
                    nc.vector.tensor_mul(new_h[lo:hi, c0:c0 + 256],
                                         s_ifo[lo:hi, 512:768], thc[lo:hi, :])
            h_tile = new_h

            if s == SMAX - 1 and smax is None:
                nc.vector.tensor_copy(hfin[96:128, :], h_tile[96:128, 384:512])
                nc.sync.dma_start(out_d[:], hfin[96:128, :])

    nc.finalize()
    return nc


def _prep_weights(inputs):
    """Host-side: transpose/permute all weights into kernel layouts (bf16)."""
    p1 = _gate_perm(H)   # 1024
    p2 = _gate_perm(E)   # 512
    w = {}
    w["ident"] = np.eye(128, dtype=BF16)
    w["ones"] = np.ones((1, 32), BF16)

    w_ih0_1 = np.asarray(inputs["w_ih0_1"], np.float32)   # [4H, F]
    w_ihr_1 = np.asarray(inputs["w_ihr_1"], np.float32)   # [NL-1, 4H, H]
    w_hh_1 = np.asarray(inputs["w_hh_1"], np.float32)     # [NL, 4H, H]
    b_1 = np.asarray(inputs["b_1"], np.float32)           # [NL, 4H]
    w_ih0_2 = np.asarray(inputs["w_ih0_2"], np.float32)   # [4E, H]
    w_ihr_2 = np.asarray(inputs["w_ihr_2"], np.float32)   # [NL-1, 4E, E]
    w_hh_2 = np.asarray(inputs["w_hh_2"], np.float32)     # [NL, 4E, E]
    b_2 = np.asarray(inputs["b_2"], np.float32)           # [NL, 4E]

    # stack1 layer 0: rows 0-7 = w.T, row 8 = bias (rides x's ones feature)
    w0 = np.empty((9, 1024), np.float32)
    w0[:8] = w_ih0_1.T[:, p1]
    w0[8] = b_1[0][p1]
    w["win0_0"] = w0.astype(BF16)
    for q in range(5):
        for k in range(2):
            w[f"whh{q}_{k}"] = np.ascontiguousarray(
                w_hh_1[q].T[128 * k:128 * (k + 1), p1]).astype(BF16)
        if q >= 1:
            for k in range(2):
                w[f"win{q}_{k}"] = np.ascontiguousarray(
                    w_ihr_1[q - 1].T[128 * k:128 * (k + 1), p1]).astype(BF16)
            w[f"bias{q}"] = b_1[q][p1][None, :].astype(BF16)

    # stack2 weights: p2-permuted [*, 512]; the slot offset is applied via
    # the psum output column (o0 = 256*c + 128*slot), not in the weights.
    for q in range(5, 10):
        l2 = q - 5
        if q == 5:
            for k in range(2):
                w[f"win{q}_{k}"] = np.ascontiguousarray(
                    w_ih0_2.T[128 * k:128 * (k + 1), p2]).astype(BF16)
        else:
            w[f"win{q}_0"] = np.ascontiguousarray(
                w_ihr_2[l2 - 1].T[:, p2]).astype(BF16)
        w[f"whh{q}_0"] = np.ascontiguousarray(
            w_hh_2[l2].T[:, p2]).astype(BF16)

    # stack2 biases: q5 solo; (6,7) pair; (8,9) pair -> [1, 1024] rows.
    def pack_bias(*qs):
        pb = np.zeros((1, 1024), np.float32)
        for q in qs:
            ba = b_2[q - 5][p2]
            slot = _slot(q)
            for c in range(4):
                pb[0, 256 * c + 128 * slot:256 * c + 128 * slot + 128] = \
                    ba[128 * c:128 * c + 128]
        return pb.astype(BF16)

    w["bias5"] = pack_bias(5)
    w["bias6"] = pack_bias(6, 7)
    w["bias8"] = pack_bias(8, 9)
    return w


def _prep_xt(x_core, T):
    """x shard [32, T, 8] -> [9, T*32] transposed with ones row (bf16)."""
    xt = np.ones((9, T * BSH), np.float32)
    xt[:8] = np.ascontiguousarray(x_core.transpose(2, 1, 0)).reshape(8, T * BSH)
    return xt.astype(BF16)


def kernel(**inputs):
    from concourse.bass_utils import run_bass_kernel_spmd

    x = np.asarray(inputs["x"], np.float32).reshape(B, T_FULL, F)
    w = _prep_weights(inputs)

    nc = build_nc(T_FULL)
    in_maps = []
    for c in range(NCORES):
        m = dict(w)
        m["xT"] = _prep_xt(x[BSH * c:BSH * (c + 1)], T_FULL)
        in_maps.append(m)
    res = run_bass_kernel_spmd(nc, in_maps, list(range(NCORES))).results
    out = np.concatenate([np.asarray(r["out"]) for r in res], axis=0)
    return out.astype(np.float32)


# revision 18
# speedup vs baseline: 51.8788x; 51.8788x over previous
"""Trainium2 Bass kernel for nn_LstmEncoder: two 5-layer LSTM stacks.

Architecture (hardcoded from the problem spec):
  x [256, 256, 8] -> stack1 (F=8 -> H=256, 5 layers) -> stack2 (H=256 -> E=128,
  5 layers) -> returns final hidden of last layer, [256, 128].

Sharding: data-parallel over batch, 32 rows per core on 8 cores; weights
replicated.  Per core the 10 layers run as a diagonal wavefront (layer q
processes t = s - q at step s); 4 layers pack into the 4 PE column groups
(tile_position col-tiling, batch=32 per group).  All matmul operands are
bf16 (1 cycle/row on TRN2 vs 4 for fp32); psum accumulation and the cell
state stay fp32.

Layouts per core:
  - Pass A psum [128, 1024]: rows 32q.. = stack1 layer q (q=0..3), cols
    [I|F|O|G] x 256.
  - Pass B psum [128, 1024]: two 512-wide column slots per 32-row block,
    each slot [I|F|O|G] x 128:
      rows 0:32  = q4 (slot0 = hidden units 0:128, slot1 = units 128:256)
      rows 32:64 = q5 (slot0); 64:96 = q6,q7; 96:128 = q8,q9.
    One N=512 matmul per (stationary, slot).  Sigmoid reads the 2-region
    AP {0:384}u{512:896}; tanh {384:512}u{896:1024}.
  - Bias: one K=4 "selector" matmul pair per pass per step writes every
    row-block's bias row into psum (start=True); gate matmuls accumulate
    onto it (BIAS_MODE="mm4").  BIAS_MODE="preload" instead copies a
    [128,1024] bias image into psum on DVE/GPSIMD and runs all matmuls
    with start=False.
  - h [128, 512] bf16 -> 4 PE transposes (bf16) -> hT stationaries.
  - Emission per step: TA(s-1) -> biasA -> phase1 MMs (pass A + q4-input)
    -> pass-A activations -> TB(s-1) -> biasB -> phase2 MMs -> pass-B
    activations, so each pass's ACT/DVE hides under the other's matmuls.
"""

import numpy as np
import ml_dtypes

B, T_FULL, F, E = 256, 256, 8, 128
H = 2 * E          # 256
NL = 5
NCORES = 8
BSH = B // NCORES  # 32

BF16 = ml_dtypes.bfloat16

# matmul emission: round-robin stride across the 4 PE column groups
EMIT_STRIDE = 1
# "mm4": K=4 selector bias matmul (standard accumulate semantics)
# "preload": engine-copied psum bias image + start=False matmuls
BIAS_MODE = "mm4"


def _gate_perm(Hd):
    """PyTorch gate order [i f g o] -> our column order [i f o g]."""
    return np.concatenate([
        np.arange(0, Hd),
        np.arange(Hd, 2 * Hd),
        np.arange(3 * Hd, 4 * Hd),
        np.arange(2 * Hd, 3 * Hd),
    ])


def _q4sel():
    """Col selector mapping p1-layout [I|F|O|G]x256 -> two slot-contiguous
    halves [I|F|O|G]x128 (units 0:128 | units 128:256)."""
    idx = []
    for s in range(2):
        for g in range(4):
            idx.extend(range(256 * g + 128 * s, 256 * g + 128 * s + 128))
    return np.array(idx)


# ---------------- layer table (chain index q = 0..9) ----------------

def _is_s1(q):
    return q < 5


def _pass_of(q):
    return "A" if q < 4 else "B"


def _rows(q):
    if q < 4:
        return 32 * q, 32 * q + 32
    r = {4: 0, 5: 32, 6: 64, 7: 64, 8: 96, 9: 96}[q]
    return r, r + 32


def _slot(q):
    return {5: 0, 6: 0, 7: 1, 8: 0, 9: 1}.get(q, 0)


def _ch_cols(q):
    if q < 4:
        return 0, 256
    if q == 4:
        return 256, 512
    return (256, 384) if _slot(q) == 0 else (384, 512)


def _own_hT(q):
    if q < 4:
        return [("A0", 32 * q), ("A1", 32 * q)]
    return {4: [("B0", 0), ("B1", 0)],
            5: [("B0", 32)],
            6: [("B0", 64)],
            7: [("B1", 64)],
            8: [("B0", 96)],
            9: [("B1", 96)]}[q]


def _in_hT(q):
    if q == 0:
        return None
    if q <= 4:
        return [("A0", 32 * (q - 1)), ("A1", 32 * (q - 1))]
    return {5: [("B0", 0), ("B1", 0)],
            6: [("B0", 32)],
            7: [("B0", 64)],
            8: [("B1", 64)],
            9: [("B0", 96)]}[q]


def _colgroup(q):
    return _rows(q)[0] // 32


def build_nc(T, smax=None, reps=1):
    import concourse.bass as bass
    import concourse.mybir as mybir
    import concourse.tile as tile
    from concourse import bacc
    from contextlib import ExitStack

    fp = mybir.dt.float32
    bf = mybir.dt.bfloat16
    AF = mybir.ActivationFunctionType
    nc = bacc.Bacc("TRN2", target_bir_lowering=False)

    # ---------------- DRAM declarations ----------------
    xT_d = nc.dram_tensor("xT", [8, T * BSH], bf, kind="ExternalInput")
    id_d = nc.dram_tensor("ident", [128, 128], bf, kind="ExternalInput")
    sel_d = nc.dram_tensor("sel4", [4, 128], bf, kind="ExternalInput")
    bA_d = nc.dram_tensor("biasA4", [4, 1024], bf, kind="ExternalInput")
    bB_d = nc.dram_tensor("biasB4", [4, 1024], bf, kind="ExternalInput")
    bAi_d = nc.dram_tensor("biasAimg", [128, 1024], bf, kind="ExternalInput")
    bBi_d = nc.dram_tensor("biasBimg", [128, 1024], bf, kind="ExternalInput")
    win_d, whh_d = {}, {}
    for q in range(10):
        G = 1024 if q <= 4 else 512
        kt_in = 1 if q == 0 else (2 if q <= 5 else 1)
        kt_hh = 2 if _is_s1(q) else 1
        for k in range(kt_in):
            kp = 8 if q == 0 else 128
            win_d[q, k] = nc.dram_tensor(f"win{q}_{k}", [kp, G], bf,
                                         kind="ExternalInput")
        for k in range(kt_hh):
            whh_d[q, k] = nc.dram_tensor(f"whh{q}_{k}", [128, G], bf,
                                         kind="ExternalInput")
    out_d = nc.dram_tensor("out", [BSH, E], fp, kind="ExternalOutput")

    with tile.TileContext(nc) as tc, ExitStack() as ctx:
        wpool = ctx.enter_context(tc.tile_pool(name="weights", bufs=1))
        state = ctx.enter_context(tc.tile_pool(name="state", bufs=1))
        hpool = ctx.enter_context(tc.tile_pool(name="hpool", bufs=2))
        hTpool = ctx.enter_context(tc.tile_pool(name="hTpool", bufs=2))
        apool = ctx.enter_context(tc.tile_pool(name="apool", bufs=2))
        gApool = ctx.enter_context(tc.tile_pool(name="gApool", bufs=2,
                                                space="PSUM"))
        gBpool = ctx.enter_context(tc.tile_pool(name="gBpool", bufs=1,
                                                space="PSUM"))
        tppool = ctx.enter_context(tc.tile_pool(name="tppool", bufs=2,
                                                space="PSUM"))

        def load(dram, dt=bf):
            t = wpool.tile(list(dram.shape), dt, name=f"sb_{dram.name}")
            nc.sync.dma_start(t[:], dram[:])
            return t

        xT_sb = load(xT_d)
        id_sb = load(id_d)
        sel_sb = load(sel_d)
        bA_sb = load(bA_d)
        bB_sb = load(bB_d)
        bAi_sb = load(bAi_d) if BIAS_MODE == "preload" else None
        bBi_sb = load(bBi_d) if BIAS_MODE == "preload" else None
        win_sb = {k: load(v) for k, v in win_d.items()}
        whh_sb = {k: load(v) for k, v in whh_d.items()}

        c_all = state.tile([128, 512], fp, name="c_all")
        nc.gpsimd.memset(c_all[:], 0.0)
        hfin = state.tile([128, 128], fp, name="hfin")

        prev_hT = None
        h_tile = None

        SMAX = (T + 9 if smax is None else smax + 1)
        for s_rep in range(reps * SMAX):
            s = s_rep % SMAX
            if s == 0:
                h_tile = None  # reps are independent runs (bench only)
            act = [q for q in range(10) if 0 <= s - q <= T - 1]
            actA = [q for q in act if _pass_of(q) == "A"]
            actB = [q for q in act if _pass_of(q) == "B"]

            for q in act:
                if s - q == 0:
                    r0, r1 = _rows(q)
                    cc0, cc1 = _ch_cols(q)
                    nc.gpsimd.memset(c_all[r0:r1, cc0:cc1], 0.0)

            g_tiles = {}
            if actA:
                g_tiles["A"] = gApool.tile([128, 1024], fp, name="gA", tag="gA")
                if BIAS_MODE == "preload":
                    nc.vector.tensor_copy(g_tiles["A"][:], bAi_sb[:])
            if actB:
                g_tiles["B"] = gBpool.tile([128, 1024], fp, name="gB", tag="gB")
                if BIAS_MODE == "preload":
                    nc.gpsimd.tensor_copy(g_tiles["B"][:], bBi_sb[:])

            def bias_mm4(pas):
                """K=4 selector matmul pair: writes all 4 row-blocks' bias."""
                g = g_tiles[pas]
                bsrc = bA_sb if pas == "A" else bB_sb
                for n0, n1 in ((0, 512), (512, 1024)):
                    nc.tensor.matmul(g[:, n0:n1], sel_sb[:, :],
                                     bsrc[:, n0:n1], start=True, stop=True,
                                     skip_group_check=True)

            def hT_ap(key, col):
                return cur_hT[key][:, col:col + 32]

            def layer_mms(q, parts):
                t = s - q
                g = g_tiles[_pass_of(q)]
                r0, r1 = _rows(q)
                mms_r = {0: [], 1: []}
                if _is_s1(q) and q != 4:
                    if "in" in parts:
                        kt_in = 1 if q == 0 else 2
                        for k in range(kt_in):
                            if q == 0:
                                lh = xT_sb[:, 32 * t:32 * t + 32]
                            else:
                                bk, col = _in_hT(q)[k]
                                lh = hT_ap(bk, col)
                            for r, (n0, n1) in enumerate(((0, 512),
                                                          (512, 1024))):
                                mms_r[r].append((g[r0:r1, n0:n1], lh,
                                                 win_sb[q, k][:, n0:n1]))
                    if "rec" in parts and t > 0:
                        for k in range(2):
                            bk, col = _own_hT(q)[k]
                            lh = hT_ap(bk, col)
                            for r, (n0, n1) in enumerate(((0, 512),
                                                          (512, 1024))):
                                mms_r[r].append((g[r0:r1, n0:n1], lh,
                                                 whh_sb[q, k][:, n0:n1]))
                elif q == 4:
                    # slot-contiguous: weight [128, 1024] q4sel order;
                    # slot s of the row block = psum cols 512s:512s+512.
                    if "in" in parts:
                        for k in range(2):
                            bk, col = _in_hT(q)[k]
                            lh = hT_ap(bk, col)
                            for r, (n0, n1) in enumerate(((0, 512),
                                                          (512, 1024))):
                                mms_r[r].append((g[r0:r1, n0:n1], lh,
                                                 win_sb[q, k][:, n0:n1]))
                    if "rec" in parts and t > 0:
                        for k in range(2):
                            bk, col = _own_hT(q)[k]
                            lh = hT_ap(bk, col)
                            for r, (n0, n1) in enumerate(((0, 512),
                                                          (512, 1024))):
                                mms_r[r].append((g[r0:r1, n0:n1], lh,
                                                 whh_sb[q, k][:, n0:n1]))
                else:
                    slot = _slot(q)
                    o0 = 512 * slot
                    if "in" in parts:
                        kt_in = 2 if q == 5 else 1
                        for k in range(kt_in):
                            bk, col = _in_hT(q)[k]
                            lh = hT_ap(bk, col)
                            mms_r[slot].append((g[r0:r1, o0:o0 + 512], lh,
                                                win_sb[q, k][:, 0:512]))
                    if "rec" in parts and t > 0:
                        bk, col = _own_hT(q)[0]
                        lh = hT_ap(bk, col)
                        mms_r[slot].append((g[r0:r1, o0:o0 + 512], lh,
                                            whh_sb[q, 0][:, 0:512]))
                seq = []
                for i in range(max(len(mms_r[0]), len(mms_r[1]))):
                    for r in range(2):
                        if i < len(mms_r[r]):
                            o, lh, rh = mms_r[r][i]
                            seq.append((o, lh, rh))
                return seq

            def emit_round_robin(groups, stride=EMIT_STRIDE):
                order = []
                if stride == 0:
                    for j in range(4):
                        order += [(j, k) for k in range(len(groups[j]))]
                else:
                    maxlen = max((len(x) for x in groups.values()), default=0)
                    for i in range(0, maxlen, stride):
                        for j in range(4):
                            for k in range(i, min(i + stride,
                                                  len(groups[j]))):
                                order.append((j, k))
                for j, k in order:
                    o, lh, rh = groups[j][k]
                    nc.tensor.matmul(o, lh, rh, start=False, stop=True,
                                     skip_group_check=True,
                                     tile_position=(0, 32 * j))

            def emit_transposes(which):
                for ki in which:
                    src = h_tile[:, 128 * ki:128 * ki + 128]
                    nc.tensor.transpose(tp_tile[:, 128 * ki:128 * ki + 128],
                                        src, id_sb[:])
                    key = ("A0", "A1", "B0", "B1")[ki]
                    dst = hTpool.tile([128, 128], bf, name=f"hT{key}",
                                      tag=f"hT{key}")
                    nc.vector.tensor_copy(dst[:],
                                          tp_tile[:, 128 * ki:128 * ki + 128])
                    cur_hT[key] = dst

            cur_hT = {}
            tp_tile = tppool.tile([128, 512], bf, name="tp", tag="tp") \
                if h_tile is not None else None
            prevA = h_tile is not None and any(0 <= (s - 1) - q <= T - 1
                                               for q in range(0, 4)) \
                and (actA or 4 in actB)
            prevB = h_tile is not None and any(0 <= (s - 1) - q <= T - 1
                                               for q in range(4, 10)) \
                and actB
            if prev_hT:
                cur_hT.update(prev_hT)

            new_h = hpool.tile([128, 512], bf, name="h", tag="h")

            # -------- pass-A state update (ACT/DVE), emitted as a closure
            def update_pass(pas, qs):
                g = g_tiles[pas]
                lo_all = min(_rows(q)[0] for q in qs)
                hi_all = max(_rows(q)[1] for q in qs)
                segs = [(32, 64), (64, hi_all)] \
                    if (lo_all == 32 and hi_all > 64) else [(lo_all, hi_all)]
                c0 = 0 if pas == "A" else 256
                s_ifo = apool.tile([128, 768], fp, name=f"sifo{pas}",
                                   tag=f"sifo{pas}")
                s_g = apool.tile([128, 256], bf, name=f"sg{pas}",
                                 tag=f"sg{pas}")
                tmp1 = apool.tile([128, 256], fp, name=f"tmp1{pas}",
                                  tag=f"tmp1{pas}")
                tmp2 = apool.tile([128, 256], fp, name=f"tmp2{pas}",
                                  tag=f"tmp2{pas}")
                thc = apool.tile([128, 256], bf, name=f"thc{pas}",
                                 tag=f"thc{pas}")

                for lo, hi in segs:
                    if pas == "A":
                        gin_s = g[lo:hi, 0:768]
                        gin_t = g[lo:hi, 768:1024]
                        f_ap = s_ifo[lo:hi, 256:512]
                        i_ap = s_ifo[lo:hi, 0:256]
                        o_ap = s_ifo[lo:hi, 512:768]
                        sg_ap = s_g[lo:hi, :]
                        t1 = tmp1[lo:hi, :]
                        t2 = tmp2[lo:hi, :]
                        thc_ap = thc[lo:hi, :]
                        c_ap = c_all[lo:hi, c0:c0 + 256]
                        h_ap = new_h[lo:hi, c0:c0 + 256]
                        sifo_out = s_ifo[lo:hi, :]
                    else:
                        # slot layout: per slot [I|F|O|G]x128
                        grr = g[lo:hi, :].rearrange("p (r c) -> p r c", r=2)
                        gin_s = grr[:, :, 0:384]
                        gin_t = grr[:, :, 384:512]
                        srr = s_ifo[lo:hi, :].rearrange("p (r c) -> p r c",
                                                        r=2)
                        sifo_out = srr
                        i_ap = srr[:, :, 0:128]
                        f_ap = srr[:, :, 128:256]
                        o_ap = srr[:, :, 256:384]
                        sg_ap = s_g[lo:hi, :].rearrange("p (r c) -> p r c",
                                                        r=2)
                        t1 = tmp1[lo:hi, :].rearrange("p (r c) -> p r c", r=2)
                        t2 = tmp2[lo:hi, :].rearrange("p (r c) -> p r c", r=2)
                        thc_ap = thc[lo:hi, :].rearrange("p (r c) -> p r c",
                                                         r=2)
                        c_ap = c_all[lo:hi, c0:c0 + 256].rearrange(
                            "p (r c) -> p r c", r=2)
                        h_ap = new_h[lo:hi, c0:c0 + 256].rearrange(
                            "p (r c) -> p r c", r=2)
                    nc.scalar.activation(sifo_out, gin_s, AF.Sigmoid)
                    nc.scalar.activation(sg_ap, gin_t, AF.Tanh)
                    nc.vector.tensor_mul(t1, f_ap, c_ap)
                    nc.vector.tensor_mul(t2, i_ap, sg_ap)
                    nc.vector.tensor_add(c_ap, t1, t2)
                    nc.scalar.activation(thc_ap, c_ap, AF.Tanh)
                    nc.vector.tensor_mul(h_ap, o_ap, thc_ap)

            # ---------------- emission ----------------
            if prevA:
                emit_transposes((0, 1))
            if actA and BIAS_MODE == "mm4":
                bias_mm4("A")
            groups1 = {j: [] for j in range(4)}
            for q in actA:
                groups1[_colgroup(q)].extend(layer_mms(q, ("in", "rec")))
            emit_round_robin(groups1)
            if actA:
                update_pass("A", actA)

            if prevB:
                emit_transposes((2, 3))
            if actB and BIAS_MODE == "mm4":
                bias_mm4("B")
            groups2 = {j: [] for j in range(4)}
            if 4 in actB:
                # q4-in must follow biasB's start=True in accumulation order
                groups2[_colgroup(4)].extend(layer_mms(4, ("in", "rec")))
            for q in actB:
                if q != 4:
                    groups2[_colgroup(q)].extend(layer_mms(q, ("in", "rec")))
            emit_round_robin(groups2)
            if actB:
                update_pass("B", actB)

            prev_hT = cur_hT
            h_tile = new_h

            if s == SMAX - 1 and smax is None:
                nc.vector.tensor_copy(hfin[96:128, :], h_tile[96:128, 384:512])
                nc.sync.dma_start(out_d[:], hfin[96:128, :])

    nc.finalize()
    return nc


def _prep_weights(inputs):
    """Host-side: transpose/permute all weights into kernel layouts (bf16)."""
    p1 = _gate_perm(H)   # 1024
    p2 = _gate_perm(E)   # 512
    q4s = _q4sel()
    w = {}
    w["ident"] = np.eye(128, dtype=BF16)
    sel = np.zeros((4, 128), np.float32)
    for k in range(4):
        sel[k, 32 * k:32 * k + 32] = 1.0
    w["sel4"] = sel.astype(BF16)

    w_ih0_1 = np.asarray(inputs["w_ih0_1"], np.float32)   # [4H, F]
    w_ihr_1 = np.asarray(inputs["w_ihr_1"], np.float32)   # [NL-1, 4H, H]
    w_hh_1 = np.asarray(inputs["w_hh_1"], np.float32)     # [NL, 4H, H]
    b_1 = np.asarray(inputs["b_1"], np.float32)           # [NL, 4H]
    w_ih0_2 = np.asarray(inputs["w_ih0_2"], np.float32)   # [4E, H]
    w_ihr_2 = np.asarray(inputs["w_ihr_2"], np.float32)   # [NL-1, 4E, E]
    w_hh_2 = np.asarray(inputs["w_hh_2"], np.float32)     # [NL, 4E, E]
    b_2 = np.asarray(inputs["b_2"], np.float32)           # [NL, 4E]

    w["win0_0"] = np.ascontiguousarray(w_ih0_1.T[:, p1]).astype(BF16)
    for q in range(4):
        for k in range(2):
            w[f"whh{q}_{k}"] = np.ascontiguousarray(
                w_hh_1[q].T[128 * k:128 * (k + 1), p1]).astype(BF16)
        if q >= 1:
            for k in range(2):
                w[f"win{q}_{k}"] = np.ascontiguousarray(
                    w_ihr_1[q - 1].T[128 * k:128 * (k + 1), p1]).astype(BF16)
    # q4: p1 then q4sel (slot-contiguous)
    for k in range(2):
        w[f"win4_{k}"] = np.ascontiguousarray(
            w_ihr_1[3].T[128 * k:128 * (k + 1), p1][:, q4s]).astype(BF16)
        w[f"whh4_{k}"] = np.ascontiguousarray(
            w_hh_1[4].T[128 * k:128 * (k + 1), p1][:, q4s]).astype(BF16)
    for q in range(5, 10):
        l2 = q - 5
        if q == 5:
            for k in range(2):
                w[f"win{q}_{k}"] = np.ascontiguousarray(
                    w_ih0_2.T[128 * k:128 * (k + 1), p2]).astype(BF16)
        else:
            w[f"win{q}_0"] = np.ascontiguousarray(
                w_ihr_2[l2 - 1].T[:, p2]).astype(BF16)
        w[f"whh{q}_0"] = np.ascontiguousarray(
            w_hh_2[l2].T[:, p2]).astype(BF16)

    # bias rows: biasA4 row a = stack1 layer a (p1); biasB4 rows:
    # 0 = q4 (p1+q4sel), 1 = q5 slot0, 2 = q6|q7, 3 = q8|q9.
    bA = np.zeros((4, 1024), np.float32)
    for a in range(4):
        bA[a] = b_1[a][p1]
    bB = np.zeros((4, 1024), np.float32)
    bB[0] = b_1[4][p1][q4s]
    bB[1, 0:512] = b_2[0][p2]
    bB[2, 0:512] = b_2[1][p2]
    bB[2, 512:1024] = b_2[2][p2]
    bB[3, 0:512] = b_2[3][p2]
    bB[3, 512:1024] = b_2[4][p2]
    w["biasA4"] = bA.astype(BF16)
    w["biasB4"] = bB.astype(BF16)
    w["biasAimg"] = np.repeat(bA, 32, axis=0).astype(BF16)
    w["biasBimg"] = np.repeat(bB, 32, axis=0).astype(BF16)
    return w


def _prep_xt(x_core, T):
    """x shard [32, T, 8] -> [8, T*32] transposed (bf16)."""
    xt = np.ascontiguousarray(
        x_core.transpose(2, 1, 0)).reshape(8, T * BSH)
    return xt.astype(BF16)


def kernel(**inputs):
    from concourse.bass_utils import run_bass_kernel_spmd

    x = np.asarray(inputs["x"], np.float32).reshape(B, T_FULL, F)
    w = _prep_weights(inputs)

    nc = build_nc(T_FULL)
    in_maps = []
    for c in range(NCORES):
        m = dict(w)
        m["xT"] = _prep_xt(x[BSH * c:BSH * (c + 1)], T_FULL)
        in_maps.append(m)
    res = run_bass_kernel_spmd(nc, in_maps, list(range(NCORES))).results
    out = np.concatenate([np.asarray(r["out"]) for r in res], axis=0)
    return out.astype(np.float32)


# revision 19
# speedup vs baseline: 52.2786x; 1.0077x over previous
"""Trainium2 Bass kernel for nn_LstmEncoder: two 5-layer LSTM stacks.

Architecture (hardcoded from the problem spec):
  x [256, 256, 8] -> stack1 (F=8 -> H=256, 5 layers) -> stack2 (H=256 -> E=128,
  5 layers) -> returns final hidden of last layer, [256, 128].

Sharding: data-parallel over batch, 32 rows per core on 8 cores; weights
replicated.  Per core the 10 layers run as a diagonal wavefront (layer q
processes t = s - q at step s); 4 layers pack into the 4 PE column groups
(tile_position col-tiling, batch=32 per group).  All matmul operands are
bf16 (1 cycle/row on TRN2 vs 4 for fp32); psum accumulation and the cell
state stay fp32.

Layouts per core:
  - Pass A psum [128, 1024]: rows 32q.. = stack1 layer q (q=0..3), cols
    [I|F|O|G] x 256.
  - Pass B psum [128, 1024]: two 512-wide column slots per 32-row block,
    each slot [I|F|O|G] x 128:
      rows 0:32  = q4 (slot0 = hidden units 0:128, slot1 = units 128:256)
      rows 32:64 = q5 (slot0); 64:96 = q6,q7; 96:128 = q8,q9.
    One N=512 matmul per (stationary, slot).  Sigmoid reads the 2-region
    AP {0:384}u{512:896}; tanh {384:512}u{896:1024}.
  - Bias: one K=4 "selector" matmul pair per pass per step writes every
    row-block's bias row into psum (start=True); gate matmuls accumulate
    onto it (BIAS_MODE="mm4").  BIAS_MODE="preload" instead copies a
    [128,1024] bias image into psum on DVE/GPSIMD and runs all matmuls
    with start=False.
  - h [128, 512] bf16 -> 4 PE transposes (bf16) -> hT stationaries.
  - Emission per step: TA(s-1) -> biasA -> phase1 MMs (pass A + q4-input)
    -> pass-A activations -> TB(s-1) -> biasB -> phase2 MMs -> pass-B
    activations, so each pass's ACT/DVE hides under the other's matmuls.
"""

import numpy as np
import ml_dtypes

B, T_FULL, F, E = 256, 256, 8, 128
H = 2 * E          # 256
NL = 5
NCORES = 8
BSH = B // NCORES  # 32

BF16 = ml_dtypes.bfloat16

# matmul emission: round-robin stride across the 4 PE column groups
# (bursts of 4 per group measured fastest on hardware)
EMIT_STRIDE = 4
# "mm4": K=4 selector bias matmul (standard accumulate semantics)
# "preload": engine-copied psum bias image + start=False matmuls
BIAS_MODE = "mm4"


def _gate_perm(Hd):
    """PyTorch gate order [i f g o] -> our column order [i f o g]."""
    return np.concatenate([
        np.arange(0, Hd),
        np.arange(Hd, 2 * Hd),
        np.arange(3 * Hd, 4 * Hd),
        np.arange(2 * Hd, 3 * Hd),
    ])


def _q4sel():
    """Col selector mapping p1-layout [I|F|O|G]x256 -> two slot-contiguous
    halves [I|F|O|G]x128 (units 0:128 | units 128:256)."""
    idx = []
    for s in range(2):
        for g in range(4):
            idx.extend(range(256 * g + 128 * s, 256 * g + 128 * s + 128))
    return np.array(idx)


# ---------------- layer table (chain index q = 0..9) ----------------

def _is_s1(q):
    return q < 5


def _pass_of(q):
    return "A" if q < 4 else "B"


def _rows(q):
    if q < 4:
        return 32 * q, 32 * q + 32
    r = {4: 0, 5: 32, 6: 64, 7: 64, 8: 96, 9: 96}[q]
    return r, r + 32


def _slot(q):
    return {5: 0, 6: 0, 7: 1, 8: 0, 9: 1}.get(q, 0)


def _ch_cols(q):
    if q < 4:
        return 0, 256
    if q == 4:
        return 256, 512
    return (256, 384) if _slot(q) == 0 else (384, 512)


def _own_hT(q):
    if q < 4:
        return [("A0", 32 * q), ("A1", 32 * q)]
    return {4: [("B0", 0), ("B1", 0)],
            5: [("B0", 32)],
            6: [("B0", 64)],
            7: [("B1", 64)],
            8: [("B0", 96)],
            9: [("B1", 96)]}[q]


def _in_hT(q):
    if q == 0:
        return None
    if q <= 4:
        return [("A0", 32 * (q - 1)), ("A1", 32 * (q - 1))]
    return {5: [("B0", 0), ("B1", 0)],
            6: [("B0", 32)],
            7: [("B0", 64)],
            8: [("B1", 64)],
            9: [("B0", 96)]}[q]


def _colgroup(q):
    return _rows(q)[0] // 32


def build_nc(T, smax=None, reps=1):
    import concourse.bass as bass
    import concourse.mybir as mybir
    import concourse.tile as tile
    from concourse import bacc
    from contextlib import ExitStack

    fp = mybir.dt.float32
    bf = mybir.dt.bfloat16
    AF = mybir.ActivationFunctionType
    nc = bacc.Bacc("TRN2", target_bir_lowering=False)

    # ---------------- DRAM declarations ----------------
    xT_d = nc.dram_tensor("xT", [8, T * BSH], bf, kind="ExternalInput")
    id_d = nc.dram_tensor("ident", [128, 128], bf, kind="ExternalInput")
    sel_d = nc.dram_tensor("sel4", [4, 128], bf, kind="ExternalInput")
    bA_d = nc.dram_tensor("biasA4", [4, 1024], bf, kind="ExternalInput")
    bB_d = nc.dram_tensor("biasB4", [4, 1024], bf, kind="ExternalInput")
    bAi_d = nc.dram_tensor("biasAimg", [128, 1024], bf, kind="ExternalInput")
    bBi_d = nc.dram_tensor("biasBimg", [128, 1024], bf, kind="ExternalInput")
    win_d, whh_d = {}, {}
    for q in range(10):
        G = 1024 if q <= 4 else 512
        kt_in = 1 if q == 0 else (2 if q <= 5 else 1)
        kt_hh = 2 if _is_s1(q) else 1
        for k in range(kt_in):
            kp = 8 if q == 0 else 128
            win_d[q, k] = nc.dram_tensor(f"win{q}_{k}", [kp, G], bf,
                                         kind="ExternalInput")
        for k in range(kt_hh):
            whh_d[q, k] = nc.dram_tensor(f"whh{q}_{k}", [128, G], bf,
                                         kind="ExternalInput")
    out_d = nc.dram_tensor("out", [BSH, E], fp, kind="ExternalOutput")

    with tile.TileContext(nc) as tc, ExitStack() as ctx:
        wpool = ctx.enter_context(tc.tile_pool(name="weights", bufs=1))
        state = ctx.enter_context(tc.tile_pool(name="state", bufs=1))
        hpool = ctx.enter_context(tc.tile_pool(name="hpool", bufs=2))
        hTpool = ctx.enter_context(tc.tile_pool(name="hTpool", bufs=2))
        apool = ctx.enter_context(tc.tile_pool(name="apool", bufs=2))
        gApool = ctx.enter_context(tc.tile_pool(name="gApool", bufs=2,
                                                space="PSUM"))
        gBpool = ctx.enter_context(tc.tile_pool(name="gBpool", bufs=1,
                                                space="PSUM"))
        tppool = ctx.enter_context(tc.tile_pool(name="tppool", bufs=2,
                                                space="PSUM"))

        def load(dram, dt=bf):
            t = wpool.tile(list(dram.shape), dt, name=f"sb_{dram.name}")
            nc.sync.dma_start(t[:], dram[:])
            return t

        xT_sb = load(xT_d)
        id_sb = load(id_d)
        sel_sb = load(sel_d)
        bA_sb = load(bA_d)
        bB_sb = load(bB_d)
        bAi_sb = load(bAi_d) if BIAS_MODE == "preload" else None
        bBi_sb = load(bBi_d) if BIAS_MODE == "preload" else None
        win_sb = {k: load(v) for k, v in win_d.items()}
        whh_sb = {k: load(v) for k, v in whh_d.items()}

        c_all = state.tile([128, 512], fp, name="c_all")
        nc.gpsimd.memset(c_all[:], 0.0)
        hfin = state.tile([128, 128], fp, name="hfin")

        prev_hT = None
        h_tile = None

        SMAX = (T + 9 if smax is None else smax + 1)
        for s_rep in range(reps * SMAX):
            s = s_rep % SMAX
            if s == 0:
                h_tile = None  # reps are independent runs (bench only)
            act = [q for q in range(10) if 0 <= s - q <= T - 1]
            actA = [q for q in act if _pass_of(q) == "A"]
            actB = [q for q in act if _pass_of(q) == "B"]

            for q in act:
                if s - q == 0:
                    r0, r1 = _rows(q)
                    cc0, cc1 = _ch_cols(q)
                    nc.gpsimd.memset(c_all[r0:r1, cc0:cc1], 0.0)

            g_tiles = {}
            if actA:
                g_tiles["A"] = gApool.tile([128, 1024], fp, name="gA", tag="gA")
                if BIAS_MODE == "preload":
                    nc.vector.tensor_copy(g_tiles["A"][:], bAi_sb[:])
            if actB:
                g_tiles["B"] = gBpool.tile([128, 1024], fp, name="gB", tag="gB")
                if BIAS_MODE == "preload":
                    nc.gpsimd.tensor_copy(g_tiles["B"][:], bBi_sb[:])

            def bias_mm4(pas):
                """K=4 selector matmul pair: writes all 4 row-blocks' bias."""
                g = g_tiles[pas]
                bsrc = bA_sb if pas == "A" else bB_sb
                for n0, n1 in ((0, 512), (512, 1024)):
                    nc.tensor.matmul(g[:, n0:n1], sel_sb[:, :],
                                     bsrc[:, n0:n1], start=True, stop=True,
                                     skip_group_check=True)

            def hT_ap(key, col):
                return cur_hT[key][:, col:col + 32]

            def layer_mms(q, parts):
                t = s - q
                g = g_tiles[_pass_of(q)]
                r0, r1 = _rows(q)
                mms_r = {0: [], 1: []}
                if _is_s1(q) and q != 4:
                    if "in" in parts:
                        kt_in = 1 if q == 0 else 2
                        for k in range(kt_in):
                            if q == 0:
                                lh = xT_sb[:, 32 * t:32 * t + 32]
                            else:
                                bk, col = _in_hT(q)[k]
                                lh = hT_ap(bk, col)
                            for r, (n0, n1) in enumerate(((0, 512),
                                                          (512, 1024))):
                                mms_r[r].append((g[r0:r1, n0:n1], lh,
                                                 win_sb[q, k][:, n0:n1]))
                    if "rec" in parts and t > 0:
                        for k in range(2):
                            bk, col = _own_hT(q)[k]
                            lh = hT_ap(bk, col)
                            for r, (n0, n1) in enumerate(((0, 512),
                                                          (512, 1024))):
                                mms_r[r].append((g[r0:r1, n0:n1], lh,
                                                 whh_sb[q, k][:, n0:n1]))
                elif q == 4:
                    # slot-contiguous: weight [128, 1024] q4sel order;
                    # slot s of the row block = psum cols 512s:512s+512.
                    if "in" in parts:
                        for k in range(2):
                            bk, col = _in_hT(q)[k]
                            lh = hT_ap(bk, col)
                            for r, (n0, n1) in enumerate(((0, 512),
                                                          (512, 1024))):
                                mms_r[r].append((g[r0:r1, n0:n1], lh,
                                                 win_sb[q, k][:, n0:n1]))
                    if "rec" in parts and t > 0:
                        for k in range(2):
                            bk, col = _own_hT(q)[k]
                            lh = hT_ap(bk, col)
                            for r, (n0, n1) in enumerate(((0, 512),
                                                          (512, 1024))):
                                mms_r[r].append((g[r0:r1, n0:n1], lh,
                                                 whh_sb[q, k][:, n0:n1]))
                else:
                    slot = _slot(q)
                    o0 = 512 * slot
                    if "in" in parts:
                        kt_in = 2 if q == 5 else 1
                        for k in range(kt_in):
                            bk, col = _in_hT(q)[k]
                            lh = hT_ap(bk, col)
                            mms_r[slot].append((g[r0:r1, o0:o0 + 512], lh,
                                                win_sb[q, k][:, 0:512]))
                    if "rec" in parts and t > 0:
                        bk, col = _own_hT(q)[0]
                        lh = hT_ap(bk, col)
                        mms_r[slot].append((g[r0:r1, o0:o0 + 512], lh,
                                            whh_sb[q, 0][:, 0:512]))
                seq = []
                for i in range(max(len(mms_r[0]), len(mms_r[1]))):
                    for r in range(2):
                        if i < len(mms_r[r]):
                            o, lh, rh = mms_r[r][i]
                            seq.append((o, lh, rh))
                return seq

            def emit_round_robin(groups, stride=EMIT_STRIDE):
                order = []
                if stride == 0:
                    for j in range(4):
                        order += [(j, k) for k in range(len(groups[j]))]
                else:
                    maxlen = max((len(x) for x in groups.values()), default=0)
                    for i in range(0, maxlen, stride):
                        for j in range(4):
                            for k in range(i, min(i + stride,
                                                  len(groups[j]))):
                                order.append((j, k))
                for j, k in order:
                    o, lh, rh = groups[j][k]
                    nc.tensor.matmul(o, lh, rh, start=False, stop=True,
                                     skip_group_check=True,
                                     tile_position=(0, 32 * j))

            def emit_transposes(which):
                for ki in which:
                    src = h_tile[:, 128 * ki:128 * ki + 128]
                    nc.tensor.transpose(tp_tile[:, 128 * ki:128 * ki + 128],
                                        src, id_sb[:])
                    key = ("A0", "A1", "B0", "B1")[ki]
                    dst = hTpool.tile([128, 128], bf, name=f"hT{key}",
                                      tag=f"hT{key}")
                    nc.vector.tensor_copy(dst[:],
                                          tp_tile[:, 128 * ki:128 * ki + 128])
                    cur_hT[key] = dst

            cur_hT = {}
            tp_tile = tppool.tile([128, 512], bf, name="tp", tag="tp") \
                if h_tile is not None else None
            prevA = h_tile is not None and any(0 <= (s - 1) - q <= T - 1
                                               for q in range(0, 4)) \
                and (actA or 4 in actB)
            prevB = h_tile is not None and any(0 <= (s - 1) - q <= T - 1
                                               for q in range(4, 10)) \
                and actB
            if prev_hT:
                cur_hT.update(prev_hT)

            new_h = hpool.tile([128, 512], bf, name="h", tag="h")

            # -------- pass-A state update (ACT/DVE), emitted as a closure
            def update_pass(pas, qs):
                g = g_tiles[pas]
                lo_all = min(_rows(q)[0] for q in qs)
                hi_all = max(_rows(q)[1] for q in qs)
                segs = [(32, 64), (64, hi_all)] \
                    if (lo_all == 32 and hi_all > 64) else [(lo_all, hi_all)]
                c0 = 0 if pas == "A" else 256
                s_ifo = apool.tile([128, 768], fp, name=f"sifo{pas}",
                                   tag=f"sifo{pas}")
                s_g = apool.tile([128, 256], bf, name=f"sg{pas}",
                                 tag=f"sg{pas}")
                tmp1 = apool.tile([128, 256], fp, name=f"tmp1{pas}",
                                  tag=f"tmp1{pas}")
                tmp2 = apool.tile([128, 256], fp, name=f"tmp2{pas}",
                                  tag=f"tmp2{pas}")
                thc = apool.tile([128, 256], bf, name=f"thc{pas}",
                                 tag=f"thc{pas}")

                for lo, hi in segs:
                    if pas == "A":
                        gin_s = g[lo:hi, 0:768]
                        gin_t = g[lo:hi, 768:1024]
                        f_ap = s_ifo[lo:hi, 256:512]
                        i_ap = s_ifo[lo:hi, 0:256]
                        o_ap = s_ifo[lo:hi, 512:768]
                        sg_ap = s_g[lo:hi, :]
                        t1 = tmp1[lo:hi, :]
                        t2 = tmp2[lo:hi, :]
                        thc_ap = thc[lo:hi, :]
                        c_ap = c_all[lo:hi, c0:c0 + 256]
                        h_ap = new_h[lo:hi, c0:c0 + 256]
                        sifo_out = s_ifo[lo:hi, :]
                    else:
                        # slot layout: per slot [I|F|O|G]x128
                        grr = g[lo:hi, :].rearrange("p (r c) -> p r c", r=2)
                        gin_s = grr[:, :, 0:384]
                        gin_t = grr[:, :, 384:512]
                        srr = s_ifo[lo:hi, :].rearrange("p (r c) -> p r c",
                                                        r=2)
                        sifo_out = srr
                        i_ap = srr[:, :, 0:128]
                        f_ap = srr[:, :, 128:256]
                        o_ap = srr[:, :, 256:384]
                        sg_ap = s_g[lo:hi, :].rearrange("p (r c) -> p r c",
                                                        r=2)
                        t1 = tmp1[lo:hi, :].rearrange("p (r c) -> p r c", r=2)
                        t2 = tmp2[lo:hi, :].rearrange("p (r c) -> p r c", r=2)
                        thc_ap = thc[lo:hi, :].rearrange("p (r c) -> p r c",
                                                         r=2)
                        c_ap = c_all[lo:hi, c0:c0 + 256].rearrange(
                            "p (r c) -> p r c", r=2)
                        h_ap = new_h[lo:hi, c0:c0 + 256].rearrange(
                            "p (r c) -> p r c", r=2)
                    nc.scalar.activation(sifo_out, gin_s, AF.Sigmoid)
                    nc.scalar.activation(sg_ap, gin_t, AF.Tanh)
                    nc.vector.tensor_mul(t1, f_ap, c_ap)
                    nc.vector.tensor_mul(t2, i_ap, sg_ap)
                    nc.vector.tensor_add(c_ap, t1, t2)
                    nc.scalar.activation(thc_ap, c_ap, AF.Tanh)
                    nc.vector.tensor_mul(h_ap, o_ap, thc_ap)

            # ---------------- emission ----------------
            if prevA:
                emit_transposes((0, 1))
            if actA and BIAS_MODE == "mm4":
                bias_mm4("A")
            groups1 = {j: [] for j in range(4)}
            for q in actA:
                groups1[_colgroup(q)].extend(layer_mms(q, ("in", "rec")))
            emit_round_robin(groups1)
            if actA:
                update_pass("A", actA)

            if prevB:
                emit_transposes((2, 3))
            if actB and BIAS_MODE == "mm4":
                bias_mm4("B")
            groups2 = {j: [] for j in range(4)}
            if 4 in actB:
                # q4-in must follow biasB's start=True in accumulation order
                groups2[_colgroup(4)].extend(layer_mms(4, ("in", "rec")))
            for q in actB:
                if q != 4:
                    groups2[_colgroup(q)].extend(layer_mms(q, ("in", "rec")))
            emit_round_robin(groups2)
            if actB:
                update_pass("B", actB)

            prev_hT = cur_hT
            h_tile = new_h

            if s == SMAX - 1 and smax is None:
                nc.vector.tensor_copy(hfin[96:128, :], h_tile[96:128, 384:512])
                nc.sync.dma_start(out_d[:], hfin[96:128, :])

    nc.finalize()
    return nc


def _prep_weights(inputs):
    """Host-side: transpose/permute all weights into kernel layouts (bf16)."""
    p1 = _gate_perm(H)   # 1024
    p2 = _gate_perm(E)   # 512
    q4s = _q4sel()
    w = {}
    w["ident"] = np.eye(128, dtype=BF16)
    sel = np.zeros((4, 128), np.float32)
    for k in range(4):
        sel[k, 32 * k:32 * k + 32] = 1.0
    w["sel4"] = sel.astype(BF16)

    w_ih0_1 = np.asarray(inputs["w_ih0_1"], np.float32)   # [4H, F]
    w_ihr_1 = np.asarray(inputs["w_ihr_1"], np.float32)   # [NL-1, 4H, H]
    w_hh_1 = np.asarray(inputs["w_hh_1"], np.float32)     # [NL, 4H, H]
    b_1 = np.asarray(inputs["b_1"], np.float32)           # [NL, 4H]
    w_ih0_2 = np.asarray(inputs["w_ih0_2"], np.float32)   # [4E, H]
    w_ihr_2 = np.asarray(inputs["w_ihr_2"], np.float32)   # [NL-1, 4E, E]
    w_hh_2 = np.asarray(inputs["w_hh_2"], np.float32)     # [NL, 4E, E]
    b_2 = np.asarray(inputs["b_2"], np.float32)           # [NL, 4E]

    w["win0_0"] = np.ascontiguousarray(w_ih0_1.T[:, p1]).astype(BF16)
    for q in range(4):
        for k in range(2):
            w[f"whh{q}_{k}"] = np.ascontiguousarray(
                w_hh_1[q].T[128 * k:128 * (k + 1), p1]).astype(BF16)
        if q >= 1:
            for k in range(2):
                w[f"win{q}_{k}"] = np.ascontiguousarray(
                    w_ihr_1[q - 1].T[128 * k:128 * (k + 1), p1]).astype(BF16)
    # q4: p1 then q4sel (slot-contiguous)
    for k in range(2):
        w[f"win4_{k}"] = np.ascontiguousarray(
            w_ihr_1[3].T[128 * k:128 * (k + 1), p1][:, q4s]).astype(BF16)
        w[f"whh4_{k}"] = np.ascontiguousarray(
            w_hh_1[4].T[128 * k:128 * (k + 1), p1][:, q4s]).astype(BF16)
    for q in range(5, 10):
        l2 = q - 5
        if q == 5:
            for k in range(2):
                w[f"win{q}_{k}"] = np.ascontiguousarray(
                    w_ih0_2.T[128 * k:128 * (k + 1), p2]).astype(BF16)
        else:
            w[f"win{q}_0"] = np.ascontiguousarray(
                w_ihr_2[l2 - 1].T[:, p2]).astype(BF16)
        w[f"whh{q}_0"] = np.ascontiguousarray(
            w_hh_2[l2].T[:, p2]).astype(BF16)

    # bias rows: biasA4 row a = stack1 layer a (p1); biasB4 rows:
    # 0 = q4 (p1+q4sel), 1 = q5 slot0, 2 = q6|q7, 3 = q8|q9.
    bA = np.zeros((4, 1024), np.float32)
    for a in range(4):
        bA[a] = b_1[a][p1]
    bB = np.zeros((4, 1024), np.float32)
    bB[0] = b_1[4][p1][q4s]
    bB[1, 0:512] = b_2[0][p2]
    bB[2, 0:512] = b_2[1][p2]
    bB[2, 512:1024] = b_2[2][p2]
    bB[3, 0:512] = b_2[3][p2]
    bB[3, 512:1024] = b_2[4][p2]
    w["biasA4"] = bA.astype(BF16)
    w["biasB4"] = bB.astype(BF16)
    w["biasAimg"] = np.repeat(bA, 32, axis=0).astype(BF16)
    w["biasBimg"] = np.repeat(bB, 32, axis=0).astype(BF16)
    return w


def _prep_xt(x_core, T):
    """x shard [32, T, 8] -> [8, T*32] transposed (bf16)."""
    xt = np.ascontiguousarray(
        x_core.transpose(2, 1, 0)).reshape(8, T * BSH)
    return xt.astype(BF16)


def kernel(**inputs):
    from concourse.bass_utils import run_bass_kernel_spmd

    x = np.asarray(inputs["x"], np.float32).reshape(B, T_FULL, F)
    w = _prep_weights(inputs)

    nc = build_nc(T_FULL)
    in_maps = []
    for c in range(NCORES):
        m = dict(w)
        m["xT"] = _prep_xt(x[BSH * c:BSH * (c + 1)], T_FULL)
        in_maps.append(m)
    res = run_bass_kernel_spmd(nc, in_maps, list(range(NCORES))).results
    out = np.concatenate([np.asarray(r["out"]) for r in res], axis=0)
    return out.astype(np.float32)
